# revision 5
# baseline (speedup 1.0000x reference)
"""Trainium2 Bass kernel v2 for the CNN-LSTM-CTC model.

Chunked-parallel LSTM: each layer's 1600-step scan is split into nch=8
time-chunks processed concurrently (folded into the batch dim:
128 columns per step instead of 16), with an L=48-step burn-in whose
outputs are discarded.  The LSTM forget gates make state influence
decay ~exponentially, so the burn-in reproduces the exact scan to
~1e-5.  Serial steps per layer drop 1600 -> 248.

Everything else (conv stack, input GEMMs, dense head) matches the v1
kernel: phase-decomposed conv GEMMs, per-layer input GEMM into HBM
(positions padded by L zeros on both ends so chunk 0 / last burn-ins
read zeros), next layer's GEMM interleaved into recurrence PE idle
windows at demoted priority.  fp16 matmul operands / fp32 PSUM.
"""
from contextlib import ExitStack
import time

import numpy as np
import jax
from jax.sharding import Mesh, PartitionSpec, NamedSharding

import bass_rust
import concourse.bass as bass
import concourse.tile as tile
import concourse.mybir as mybir
import concourse.bass2jax as b2j

# ======================================================================
# geometry / host-side prep
# ======================================================================
B = 16          # per-core batch
T = 8000
U = 1600        # output timesteps
J = 1605        # tau blocks (tau = 5j + p)
NJ = J * B      # conv moving columns, col = j*16 + b
H = 384
G4 = 1536

NCH = 8         # time chunks folded into batch
CH = U // NCH   # 200 steps per chunk
L = 16          # burn-in steps
S = CH + L      # recurrence steps per layer (224)
NB = NCH * B    # recurrence batch columns (128)
UB = 8          # ring/ublock size (S == 28*UB, L == 3*UB)
U2 = U + 2 * L  # padded xgb position count

WORKER_PRIO = -100_000_000   # scheduler priority offset for worker ops
EVAC_MODE = "dve"            # og evacuation: "act" | "dve" | "mix"
INJ_MODE = "pe"              # xg injection: "pe" | "dve" (i,f,g via DVE add)

# r-permutation for c2out partitions: position of r in the partition order
R_ORDER = [3, 4, 0, 1, 2]            # partition block -> r
R_POS = {r: i for i, r in enumerate(R_ORDER)}   # r -> partition block
# conv3 q' groups: (q', [dk3 list], partition range)
Q_GROUPS = []
for q in range(5):
    dks = [dk for dk in range(19) if (dk + 3) // 5 == q]
    rows = sorted(R_POS[(dk + 3) % 5] for dk in dks)
    lo, hi = rows[0] * 16, (rows[-1] + 1) * 16
    Q_GROUPS.append((q, dks, lo, hi))
# -> q0: rows [0:32), q1-3: [0:80), q4: [32:64)


def f16(a):
    return np.ascontiguousarray(a, dtype=np.float16)


def f32(a):
    return np.ascontiguousarray(a, dtype=np.float32)


def prep_weights(inputs):
    """Prepare all weight/bias tensors (shared across cores)."""
    w1, b1 = np.asarray(inputs['conv1_w']), np.asarray(inputs['conv1_b'])
    w2, b2 = np.asarray(inputs['conv2_w']), np.asarray(inputs['conv2_b'])
    w3, b3 = np.asarray(inputs['conv3_w']), np.asarray(inputs['conv3_b'])
    Wi, Wh, bl = (np.asarray(inputs['lstm_Wi']), np.asarray(inputs['lstm_Wh']),
                  np.asarray(inputs['lstm_b']))
    dw, db = np.asarray(inputs['dense_w']), np.asarray(inputs['dense_b'])

    out = {}
    # conv1: lhsT1 [9, 20]; M order (p, co): idx = p*4+co
    lhsT1 = np.zeros((9, 20), np.float32)
    for o in range(9):
        for p in range(5):
            dk = o - p
            if 0 <= dk < 5:
                lhsT1[o, p * 4:(p + 1) * 4] = w1[dk, 0, :]
    out['lhsT1'] = f16(lhsT1)
    out['b1c'] = f32(np.tile(b1, 5))           # [20] bias per (p,co)

    # conv2: lhsT2 [36, 80]; K order (r, ci): idx = r*4+ci;
    # M order (R_POS[p], co): idx = R_POS[p]*16+co
    lhsT2 = np.zeros((36, 80), np.float32)
    for r in range(9):
        for ci in range(4):
            for p in range(5):
                dk = r - p
                if 0 <= dk < 5:
                    lhsT2[r * 4 + ci, R_POS[p] * 16:(R_POS[p] + 1) * 16] = \
                        w2[dk, ci, :]
    out['lhsT2'] = f16(lhsT2)
    b2c = np.zeros(80, np.float32)
    for p in range(5):
        b2c[R_POS[p] * 16:(R_POS[p] + 1) * 16] = b2
    out['b2c'] = f32(b2c)

    # conv3: lhsT3 [80, 5, 384]: [(rpos,ci), q', col] = w3[dk3, ci, col]
    lhsT3 = np.zeros((80, 5, 384), np.float32)
    for q, dks, lo, hi in Q_GROUPS:
        for dk in dks:
            rpos = R_POS[(dk + 3) % 5]
            for ci in range(16):
                lhsT3[rpos * 16 + ci, q, :] = w3[dk, ci, :]
    out['lhsT3'] = f16(lhsT3)
    out['b3c'] = f32(b3.reshape(3, 128))

    # lstm: WiT/WhT [5, 3, 12, 128(r), 128(c)] (compute layout)
    out['WiT'] = f16(Wi.reshape(5, 3, 128, 12, 128).transpose(0, 1, 3, 2, 4))
    out['WhT'] = f16(Wh.reshape(5, 3, 128, 12, 128).transpose(0, 1, 3, 2, 4))
    # device layouts [5, 128(r), 3, 12, 128(c)] so the DMA is contiguous
    out['WiT_dev'] = f16(out['WiT'].transpose(0, 3, 1, 2, 4))
    out['WhT_dev'] = f16(out['WhT'].transpose(0, 3, 1, 2, 4))
    out['bLc'] = f32(bl.reshape(5, 12, 128))

    # dense: dWT [3, 128, 5]; device layout [128, 3, 5]
    out['dWT'] = f16(dw.reshape(3, 128, 5))
    out['dWT_dev'] = f16(dw.reshape(3, 128, 5).transpose(1, 0, 2))
    out['dbc'] = f32(db)                       # [5]
    return out


def prep_x(x_core):
    """x_core: [B, T] float32 -> rhs1 [9, NJ] float16.
    rhs1[o, j*16+b] = x[b, 5j + o - 12]  (0 outside [0,T))."""
    xb = np.asarray(x_core, dtype=np.float32)
    assert xb.shape == (B, T)
    rhs1 = np.zeros((9, J, B), np.float32)
    for o in range(9):
        # t index = 5j + o - 12 for j in [0, J)
        tidx = 5 * np.arange(J) + o - 12
        valid = (tidx >= 0) & (tidx < T)
        rhs1[o, valid, :] = xb[:, tidx[valid]].T
    return f16(rhs1.reshape(9, NJ))


# ======================================================================
# BIR post-processing (walrus supports only 1 sem wait per instruction)
# ======================================================================
MAX_WAITS = 1


def split_excess_waits(nc, max_waits=MAX_WAITS):
    """Walrus codegen only supports `max_waits` semaphore waits per
    instruction; tile's tail drain (and occasionally other instructions)
    can carry more. Hoist the excess into preceding same-engine NoOps."""
    n_fixed = 0
    for fn in nc.m.functions:
        for b in fn.blocks:
            insts = b.instructions
            out = []
            dirty = False
            for inst in insts:
                si = inst.sync_info
                if si is not None and len(si.on_wait) > max_waits:
                    waits = list(si.on_wait)
                    extra, keep = waits[:-max_waits], waits[-max_waits:]
                    k = 0
                    while extra:
                        chunk, extra = extra[:max_waits], extra[max_waits:]
                        nop = mybir.InstNoOp(
                            name=f"{inst.name}_waitsplit{k}", ins=[], outs=[])
                        nop.engine = inst.engine
                        nop.sync_info = bass_rust.SyncInfo(
                            on_wait=chunk, on_update=[])
                        out.append(nop)
                        k += 1
                    si.on_wait = keep
                    n_fixed += 1
                    dirty = True
                out.append(inst)
            if dirty:
                b.instructions = out
    return n_fixed


# ======================================================================
# bass/tile program builder
# ======================================================================
DT16 = mybir.dt.float16
DT32 = mybir.dt.float32
AF = mybir.ActivationFunctionType
ALU = mybir.AluOpType


def perm(ap, order):
    """Permute the dims of an AP, dropping unit dims (DRAM-side APs only)."""
    dims = [ap.ap[i] for i in order]
    dims = [d for d in dims if d[1] != 1] or dims[:1]
    return bass.AP(tensor=ap.tensor, offset=ap.offset, ap=dims)


class GemmEmitter:
    """Emits the xg GEMM for layer `l`, writing xgb (position-padded)."""

    def __init__(self, tc, nc, l, xin, xgb, WiT_d, bL_d):
        self.tc, self.nc, self.l = tc, nc, l
        self.xin, self.xgb = xin, xgb
        self.ctx = ExitStack()
        self.wp = self.ctx.enter_context(tc.tile_pool(name=f"gw{l}", bufs=1))
        self.io = self.ctx.enter_context(tc.tile_pool(name=f"gio{l}", bufs=3))
        self.ps = self.ctx.enter_context(
            tc.tile_pool(name=f"gps{l}", bufs=2, space="PSUM"))
        self.WiT_s = self.wp.tile([128, 3, 12, 128], DT16, tag=f"giw{l}")
        nc.sync.dma_start(self.WiT_s[:], WiT_d[l])
        self.bL_s = self.wp.tile([128, 12], DT32, tag=f"gib{l}")
        nc.sync.dma_start(self.bL_s[:], bL_d[:, l, :])

    def emit_block(self, rx, uo, nu):
        """Consume one conv3 output block rx [128, 3, nu*B] (positions
        uo..uo+nu), write xgb0 at idx+L."""
        nc = self.nc
        n = nu * B
        for m in range(12):
            pg = self.ps.tile([128, 512], DT32, tag=f"gpg{self.l}")
            for k in range(3):
                nc.tensor.matmul(pg[:, :n], self.WiT_s[:, k, m, :],
                                 rx[:, k, :n], start=(k == 0), stop=(k == 2))
            og = self.io.tile([128, 512], DT16, tag=f"gog{self.l}")
            nc.scalar.activation(og[:, :n], pg[:, :n], AF.Identity,
                                 bias=self.bL_s[:, m:m + 1])
            nc.sync.dma_start(self.xgb[:, m, L + uo:L + uo + nu, :],
                              og[:, :n])

    def emit_ring(self, ring, s0, parent_reverse):
        """Consume a full ring ublock (all slots valid: s0 >= L) and write
        the next layer's chunk-folded xgb [128, 12, S, NB].

        Ring slot w holds step s_l = s0 + UB-1-w, so ascending w maps to
        ascending consumer slot s' = C+2L-1 - s_l.  Columns carry over
        1:1 (chunk identity alternates per layer); the first L steps of
        each chunk are additionally duplicated into the next chunk's
        burn-in slots, shifted one column block."""
        nc = self.nc
        cs = 4                       # ring slots per 512-col GEMM chunk
        for w0 in range(0, UB, cs):
            oga = self.io.tile([128, 12, cs, NB], DT16, tag=f"goga{self.l}")
            for m in range(12):
                pg = self.ps.tile([128, 512], DT32, tag=f"gpg{self.l}")
                for k in range(3):
                    nc.tensor.matmul(pg[:], self.WiT_s[:, k, m, :],
                                     ring[:, w0:w0 + cs, k, :],
                                     start=(k == 0), stop=(k == 2))
                use_act = (EVAC_MODE == "act" or
                           (EVAC_MODE == "mix" and m % 2 == 0))
                if use_act:
                    nc.scalar.activation(oga[:, m, :, :], pg[:], AF.Identity,
                                         bias=self.bL_s[:, m:m + 1])
                else:
                    nc.vector.tensor_scalar_add(oga[:, m, :, :], pg[:],
                                                self.bL_s[:, m:m + 1])
            lo = (CH + 2 * L - UB) - s0 + w0     # s' of slot w0
            nc.sync.dma_start(self.xgb[:, :, lo:lo + cs, :], oga[:])

    def close(self):
        self.ctx.close()


class DenseEmitter:
    """Emits the dense CTC head from layer-4 rings."""

    def __init__(self, tc, nc, dWT_d, db_d, y_d):
        self.tc, self.nc = tc, nc
        self.y_d = y_d
        self.ctx = ExitStack()
        self.wp = self.ctx.enter_context(tc.tile_pool(name="dw", bufs=1))
        self.io = self.ctx.enter_context(tc.tile_pool(name="dio", bufs=3))
        self.ps = self.ctx.enter_context(
            tc.tile_pool(name="dps", bufs=2, space="PSUM"))
        self.dW_s = self.wp.tile([128, 3, 5], DT16, tag="dwt")
        nc.sync.dma_start(self.dW_s[:], dWT_d[:])
        self.db_s = self.wp.tile([5, 1], DT32, tag="dbt")
        nc.sync.dma_start(self.db_s[:], db_d[:])

    def emit_ring(self, ring, s0, parent_reverse):
        # parent is layer 4 (reverse, chunk==column): ring slot w holds
        # s_l = s0+UB-1-w -> t = 1599 - (q*CH - L + s_l), ascending in w.
        nc = self.nc
        cs = 4
        for w0 in range(0, UB, cs):
            pd = self.ps.tile([5, 512], DT32, tag="pd")
            for k in range(3):
                nc.tensor.matmul(pd[:], self.dW_s[:, k, :],
                                 ring[:, w0:w0 + cs, k, :],
                                 start=(k == 0), stop=(k == 2))
            oy = self.io.tile([5, 4, NB], DT32, tag="oy")
            nc.scalar.activation(oy[:], pd[:], AF.Identity, bias=self.db_s[:])
            for q in range(NCH):
                lo = (U + L - UB) - q * CH - s0 + w0
                nc.sync.dma_start(self.y_d[:, lo:lo + cs, :],
                                  oy[:, :, q * B:(q + 1) * B])

    def close(self):
        self.ctx.close()


def build(layers=5, with_conv=True):
    nc = bass.Bass("TRN2", target_bir_lowering=False, debug=False)

    def din(name, shape, dt=DT16):
        return nc.dram_tensor(name, shape, dt, kind="ExternalInput").ap()

    def scratch(name, shape, dt=DT16):
        return nc.dram_tensor(name, shape, dt, kind="Internal").ap()

    if with_conv:
        rhs1_d = din("rhs1", [9, NJ])
        lhsT1_d = din("lhsT1", [9, 20])
        b1_d = din("b1c", [20, 1], DT32)
        lhsT2_d = din("lhsT2", [36, 80])
        b2_d = din("b2c", [80, 1], DT32)
        lhsT3_d = din("lhsT3", [80, 5, 384])
        b3_d = din("b3c", [128, 3], DT32)
    else:
        xt0_d = din("xt0", [3, 128, U, B])
    if layers:
        ident_d = din("ident", [128, 128])
        WiT_d = din("WiT", [layers, 128, 3, 12, 128])
        WhT_d = din("WhT", [layers, 128, 3, 12, 128])
        bL_d = din("bLc", [128, layers, 12], DT32)
    dWT_d = din("dWT", [128, 3, 5])
    db_d = din("dbc", [5, 1], DT32)
    y_d = nc.dram_tensor("y", [5, U, B], DT32, kind="ExternalOutput").ap()

    # layer 0: absolute-position layout (written by emit_cols);
    # layers 1-4: chunk-folded layouts in consumer processing coords,
    # alternating between two buffers (A: layers 1,3; B: layers 2,4).
    xgb0 = scratch("xgbs0", [128, 12, U2, B])
    xgbA = scratch("xgbsA", [128, 12, S, NB])
    xgbB = scratch("xgbsB", [128, 12, S, NB])

    with tile.TileContext(nc) as tc:
        # zero the burn-in position pads of the layer-0 buffer once
        with tc.tile_pool(name="zp", bufs=1) as zp:
            ztile = zp.tile([128, 12, L, B], DT16)
            nc.vector.memset(ztile[:], 0)
            nc.sync.dma_start(xgb0[:, :, 0:L, :], ztile[:])
            nc.sync.dma_start(xgb0[:, :, U + L:U2, :], ztile[:])

        if layers:
            g0 = GemmEmitter(tc, nc, 0, None, xgb0, WiT_d, bL_d)
            conv_stage(tc, nc, U, 30, rhs1_d, lhsT1_d, b1_d, lhsT2_d,
                       b2_d, lhsT3_d, b3_d, g0)
            g0.close()

        # recurrence state shared across layers (parity-tagged) so no
        # pool-lifetime barriers sit between consecutive layers
        rs = ExitStack()
        rw = rs.enter_context(tc.tile_pool(name="rw", bufs=1))
        ep = rs.enter_context(tc.tile_pool(name="rep", bufs=2))
        zp = rs.enter_context(tc.tile_pool(name="rz", bufs=1, space="PSUM"))
        shared = {
            'ident': rw.tile([128, 128], DT16, tag="ident", name="identt"),
            'hzero': rw.tile([128, 3, NB], DT16, tag="hzero", name="hzerot"),
            'ep': ep,
            'pz': [zp.tile([128, 3, NB], DT32, tag=t, name=t + "t")
                   for t in ("pzi", "pzf", "pzg", "pzo")],
        }
        for p in range(2):
            shared[f'wp{p}'] = rw
            shared[f'WhT{p}'] = rw.tile([128, 3, 12, 128], DT16,
                                        tag=f"WhT{p}", name=f"WhT{p}t")
            shared[f'c{p}'] = rw.tile([128, 3, NB], DT16, tag=f"c{p}",
                                      name=f"c{p}t")
            shared[f'rings{p}'] = [
                rw.tile([128, UB, 3, NB], DT16, tag=f"ring{p}{i}",
                        name=f"ring{p}{i}t")
                for i in range(2)]
            shared[f'xp{p}'] = rs.enter_context(
                tc.tile_pool(name=f"rxg{p}", bufs=2))

        xgb_next = [None, xgbA, xgbB, xgbA, xgbB]
        for l in range(layers):
            if l + 1 < layers:
                nxt = GemmEmitter(tc, nc, l + 1, None, xgb_next[l + 1],
                                  WiT_d, bL_d)
            else:
                nxt = DenseEmitter(tc, nc, dWT_d, db_d, y_d)
            recurrence(tc, nc, l, xgb0 if l == 0 else xgb_next[l],
                       WhT_d, ident_d, reverse=(l % 2 == 0), worker=nxt,
                       abs_xg=(l == 0), shared=shared)
            nxt.close()
        rs.close()
        if not layers:
            d = DenseEmitter(tc, nc, dWT_d, db_d, y_d)
            d.close()

    return nc


def conv_stage(tc, nc, u_steps, cb_j, rhs1_d, lhsT1_d, b1_d, lhsT2_d,
               b2_d, lhsT3_d, b3_d, g0):
    """Conv stack with SBUF-resident intermediates; conv3 output blocks
    are consumed directly by the layer-0 input GEMM (g0.emit_block)."""
    ctx = ExitStack()
    wp = ctx.enter_context(tc.tile_pool(name="cw", bufs=1))
    io = ctx.enter_context(tc.tile_pool(name="cio", bufs=3))
    ps = ctx.enter_context(tc.tile_pool(name="cps", bufs=2, space="PSUM"))

    lhsT1_s = wp.tile([9, 20], DT16)
    nc.sync.dma_start(lhsT1_s[:], lhsT1_d[:])
    b1_s = wp.tile([20, 1], DT32)
    nc.sync.dma_start(b1_s[:], b1_d[:])
    lhsT2_s = wp.tile([36, 80], DT16)
    nc.sync.dma_start(lhsT2_s[:], lhsT2_d[:])
    b2_s = wp.tile([80, 1], DT32)
    nc.sync.dma_start(b2_s[:], b2_d[:])
    lhsT3_s = wp.tile([80, 5, 384], DT16)
    nc.sync.dma_start(lhsT3_s[:], lhsT3_d[:])
    b3_s = wp.tile([128, 3], DT32)
    nc.sync.dma_start(b3_s[:], b3_d[:])
    c1_s = wp.tile([20, J + 2, B], DT16)
    c2_s = wp.tile([80, J, B], DT16)

    CB = cb_j * B
    jblocks = [(j0, min(cb_j, J - j0)) for j0 in range(0, J, cb_j)]
    for j0, nj in jblocks:
        n = nj * B
        r1 = io.tile([9, CB], DT16, tag="r1")
        nc.sync.dma_start(r1[:, :n], rhs1_d[:, j0 * B:j0 * B + n])
        p1 = ps.tile([20, CB], DT32, tag="p1")
        nc.tensor.matmul(p1[:, :n], lhsT1_s[:], r1[:, :n], start=True, stop=True)
        sg1 = io.tile([20, CB], DT32, tag="sg1")
        nc.scalar.activation(sg1[:, :n], p1[:, :n], AF.Sigmoid, bias=b1_s[:])
        nc.vector.scalar_tensor_tensor(c1_s[:, j0 + 1:j0 + 1 + nj, :],
                                       p1[:, :n], b1_s[:],
                                       sg1[:, :n], op0=ALU.add, op1=ALU.mult)
    nc.vector.memset(c1_s[:, 0:3, :], 0)
    nc.vector.memset(c1_s[:, J - 2:J + 2, :], 0)

    for j0, nj in jblocks:
        n = nj * B
        r2 = io.tile([36, CB], DT16, tag="r2")
        nc.sync.dma_start(r2[0:8, :n], c1_s[12:20, j0:j0 + nj, :])
        nc.sync.dma_start(r2[8:28, :n], c1_s[0:20, j0 + 1:j0 + 1 + nj, :])
        nc.sync.dma_start(r2[28:36, :n], c1_s[0:8, j0 + 2:j0 + 2 + nj, :])
        p2 = ps.tile([80, CB], DT32, tag="p2")
        nc.tensor.matmul(p2[:, :n], lhsT2_s[:], r2[:, :n],
                         start=True, stop=True)
        sg2 = io.tile([80, CB], DT32, tag="sg2")
        nc.scalar.activation(sg2[:, :n], p2[:, :n], AF.Sigmoid, bias=b2_s[:])
        nc.vector.scalar_tensor_tensor(c2_s[:, j0:j0 + nj, :], p2[:, :n],
                                       b2_s[:], sg2[:, :n],
                                       op0=ALU.add, op1=ALU.mult)
    nc.vector.memset(c2_s[:, 0:2, :], 0)
    nc.vector.memset(c2_s[:, J - 3:J, :], 0)

    UC = 32
    # descending t so the reversed layer 0 can start consuming early
    for u0 in reversed(range(0, u_steps, UC)):
        nu = min(UC, u_steps - u0)
        rx3 = io.tile([128, 3, UC * B], DT16, tag="rx3")
        for m in range(3):
            p3 = ps.tile([128, UC * B], DT32, tag="p3")
            for q, _dks, lo, hi in Q_GROUPS:
                nc.tensor.matmul(
                    p3[:, :nu * B],
                    lhsT3_s[lo:hi, q, m * 128:(m + 1) * 128],
                    c2_s[lo:hi, u0 + q:u0 + q + nu, :],
                    start=(q == 0), stop=(q == 4))
            sg3 = io.tile([128, UC * B], DT32, tag="sg3")
            nc.scalar.activation(sg3[:, :nu * B], p3[:, :nu * B], AF.Sigmoid,
                                 bias=b3_s[:, m:m + 1])
            nc.vector.scalar_tensor_tensor(rx3[:, m, :nu * B], p3[:, :nu * B],
                                           b3_s[:, m:m + 1], sg3[:, :nu * B],
                                           op0=ALU.add, op1=ALU.mult)
        g0.emit_block(rx3, u0, nu)
    ctx.close()


def recurrence(tc, nc, l, xgb_hbm, WhT_d, ident_d, reverse, worker=None,
               abs_xg=False, shared=None):
    """Chunked recurrence: S steps x NB columns.  Gate groups i,f,g,o in
    four single-bank PSUM tiles; xg injected via identity matmuls.

    Ring slot convention (all layers): step sl of a ublock writes h into
    slot UB-1-sl, so og/dense scatter DMAs are ascending.  xg tiles:
    chunk-folded layers load one [128,12,UB,NB] block (slot = sl);
    abs_xg (layer 0) gathers per chunk from the absolute-position buffer
    (slot = UB-1-sl for reverse)."""
    # shared tiles/pools persist across layers so consecutive layers'
    # instructions can overlap (a fresh per-layer PSUM pool would force a
    # full serialization barrier between layers -- only 8 banks exist).
    # Per-layer state alternates via the parity tags inside `shared`.
    sp = shared
    par_l = l % 2
    wp = sp[f'wp{par_l}']
    WhT_s = sp[f'WhT{par_l}']
    nc.sync.dma_start(WhT_s[:], WhT_d[l])
    ident_s = sp['ident']
    if l == 0:
        nc.sync.dma_start(ident_s[:], ident_d[:])
    c_s = sp[f'c{par_l}']
    nc.vector.memset(c_s[:], 0)
    hzero = sp['hzero']
    if l == 0:
        nc.vector.memset(hzero[:], 0)
    rings = sp[f'rings{par_l}']
    xp = sp[f'xp{par_l}']
    ep = sp['ep']
    c2 = ExitStack()

    pzi, pzf, pzg, pzo = sp['pz']
    # gate order in the m dim: i(0:3) f(3:6) g(6:9) o(9:12)
    groups = [(pzg, 6), (pzi, 0), (pzf, 3), (pzo, 9)]

    nblocks = S // UB
    for blk in range(nblocks):
        par = blk % 2
        ring = rings[par]
        s0 = blk * UB
        if abs_xg:
            # j-separated staging: [128, 12, NCH, UB, B] keeps each
            # per-chunk DMA's destination contiguous (256B runs)
            xg_s = xp.tile([128, 12, NCH, UB, B], DT16, tag=f"xg{par_l}")
            for j in range(NCH):
                if not reverse:
                    lo = j * CH + s0
                else:
                    lo = U2 - j * CH - s0 - UB
                nc.sync.dma_start(xg_s[:, :, j, :, :],
                                  xgb_hbm[:, :, lo:lo + UB, :])
        elif s0 >= L:
            xg_s = xp.tile([128, 12, UB, NB], DT16, tag=f"xg{par_l}")
            nc.sync.dma_start(xg_s[:], xgb_hbm[:, :, s0:s0 + UB, :])
        else:
            # burn-in slots hold the same xg as main slots at s+CH with
            # the columns shifted one chunk block; the boundary block is
            # chunk 0's pre-sequence region (zeros).
            xg_s = xp.tile([128, 12, UB, NB], DT16, tag=f"xg{par_l}")
            if reverse:     # this layer's chunks are column-reversed
                nc.sync.dma_start(xg_s[:, :, :, B:NB],
                                  xgb_hbm[:, :, s0 + CH:s0 + CH + UB, 0:NB - B])
                nc.vector.memset(xg_s[:, :, :, 0:B], 0)
            else:
                nc.sync.dma_start(xg_s[:, :, :, 0:NB - B],
                                  xgb_hbm[:, :, s0 + CH:s0 + CH + UB, B:NB])
                nc.vector.memset(xg_s[:, :, :, NB - B:NB], 0)
        for sl in range(UB):
            s = s0 + sl
            wslot = UB - 1 - sl
            if s == 0:
                hp = hzero
            elif sl == 0:
                hp = rings[1 - par][:, 0, :, :]
            else:
                hp = ring[:, wslot + 1, :, :]
            if abs_xg:
                slot = (UB - 1 - sl) if reverse else sl
                xg_sl = [xg_s[:, g0:g0 + 3, :, slot, :] for _, g0 in groups]
            else:
                xg_sl = [xg_s[:, g0:g0 + 3, sl, :] for _, g0 in groups]
            # xg injection (doesn't depend on h -> fills tail idle)
            pe_inj = groups if INJ_MODE == "pe" else groups[3:]
            for (dst, g0), xsl in zip(groups, xg_sl):
                if any(d is dst for d, _ in pe_inj):
                    nc.tensor.matmul(dst[:], ident_s[:], xsl,
                                     start=True, stop=False,
                                     skip_group_check=True)
            # h @ Wh accumulation, group-major (g, i, f, o)
            first = INJ_MODE != "pe"
            for dst, g0 in groups:
                st = first and not any(d is dst for d, _ in pe_inj)
                for u in range(3):
                    for k in range(3):
                        nc.tensor.matmul(dst[:, u, :], WhT_s[:, k, g0 + u, :],
                                         hp[:, k, :], start=(st and k == 0),
                                         stop=(k == 2), skip_group_check=True)
            # gate math (fp16); with INJ_MODE=dve the i/f/g xg terms are
            # added by DVE after the matmuls instead of PE injection
            if INJ_MODE == "pe":
                zg, zi, zf = pzg, pzi, pzf
            else:
                zg = ep.tile([128, 3, NB], DT32, tag="zg")
                nc.vector.tensor_add(zg[:], pzg[:], xg_sl[0])
                zi = ep.tile([128, 3, NB], DT32, tag="zi")
                nc.vector.tensor_add(zi[:], pzi[:], xg_sl[1])
                zf = ep.tile([128, 3, NB], DT32, tag="zf")
                nc.vector.tensor_add(zf[:], pzf[:], xg_sl[2])
            sg = ep.tile([128, 3, NB], DT16, tag="sg")
            nc.scalar.activation(sg[:], zg[:], AF.Tanh)
            si = ep.tile([128, 3, NB], DT16, tag="si")
            nc.scalar.activation(si[:], zi[:], AF.Sigmoid)
            ig = ep.tile([128, 3, NB], DT16, tag="ig")
            nc.vector.tensor_mul(ig[:], si[:], sg[:])
            sf = ep.tile([128, 3, NB], DT16, tag="sf")
            nc.scalar.activation(sf[:], zf[:], AF.Sigmoid)
            so = ep.tile([128, 3, NB], DT16, tag="so")
            nc.scalar.activation(so[:], pzo[:], AF.Sigmoid)
            nc.vector.tensor_mul(c_s[:], sf[:], c_s[:])
            nc.vector.tensor_add(c_s[:], c_s[:], ig[:])
            tc_ = ep.tile([128, 3, NB], DT16, tag="tc")
            nc.scalar.activation(tc_[:], c_s[:], AF.Tanh)
            nc.vector.tensor_mul(ring[:, wslot, :, :], so[:], tc_[:])
        if worker is not None and s0 >= L:
            with tc.high_priority(offset=WORKER_PRIO):
                worker.emit_ring(ring, s0, reverse)
    c2.close()


# ======================================================================
# PJRT SPMD runner
# ======================================================================
P = PartitionSpec


class SpmdRunner:
    def __init__(self, nc, n_cores=8):
        b2j.install_neuronx_cc_hook()
        self.nc = nc
        partition_name = (nc.partition_id_tensor.name
                          if nc.partition_id_tensor else None)
        self.n_cores = n_cores
        in_names, out_names, out_avals, zero_outs = [], [], [], []
        for alloc in nc.m.functions[0].allocations:
            if not isinstance(alloc, mybir.MemoryLocationSet):
                continue
            name = alloc.memorylocations[0].name
            if alloc.kind == "ExternalInput":
                if name != partition_name:
                    in_names.append(name)
            elif alloc.kind == "ExternalOutput":
                shape = tuple(alloc.tensor_shape)
                dtype = mybir.dt.np(alloc.dtype)
                out_names.append(name)
                out_avals.append(jax.core.ShapedArray(shape, dtype))
                zero_outs.append(np.zeros(shape, dtype))
        self.in_names, self.out_names = in_names, out_names
        self.out_avals, self.zero_outs = out_avals, zero_outs
        n_params = len(in_names)
        all_names = in_names + out_names
        if partition_name is not None:
            all_names = all_names + [partition_name]
        all_names = tuple(all_names)

        def _body(*args):
            operands = list(args)
            if partition_name is not None:
                operands.append(b2j.partition_id_tensor())
            outs = b2j._bass_exec_p.bind(
                *operands,
                out_avals=tuple(out_avals),
                in_names=all_names,
                out_names=tuple(out_names),
                lowering_input_output_aliases=(),
                sim_require_finite=True,
                sim_require_nnan=True,
                nc=nc,
            )
            return tuple(outs)

        devices = jax.devices()[:n_cores]
        self.mesh = Mesh(np.asarray(devices), ("core",))
        self.sharding = NamedSharding(self.mesh, P("core"))
        n_outs = len(out_names)
        self.fn = jax.jit(
            jax.shard_map(_body, mesh=self.mesh,
                          in_specs=(P("core"),) * (n_params + n_outs),
                          out_specs=(P("core"),) * n_outs,
                          check_vma=False),
            donate_argnums=tuple(range(n_params, n_params + n_outs)),
            keep_unused=True,
        )

    def place_inputs(self, in_maps):
        concat = [np.concatenate([np.asarray(m[n]) for m in in_maps], axis=0)
                  for n in self.in_names]
        return [jax.device_put(a, self.sharding) for a in concat]

    def _zeros(self):
        return [jax.device_put(
            np.zeros((self.n_cores * z.shape[0], *z.shape[1:]), z.dtype),
            self.sharding) for z in self.zero_outs]

    def run(self, in_dev, time_reps=0):
        """Returns (per-core outputs list, exec_seconds or None)."""
        out = self.fn(*in_dev, *self._zeros())
        jax.block_until_ready(out)
        best = None
        for _ in range(time_reps):
            zs = self._zeros()
            jax.block_until_ready(zs)
            jax.block_until_ready(in_dev)
            t0 = time.perf_counter()
            out = self.fn(*in_dev, *zs)
            jax.block_until_ready(out)
            dt = time.perf_counter() - t0
            best = dt if best is None else min(best, dt)
        results = []
        for c in range(self.n_cores):
            results.append({
                n: np.asarray(out[i]).reshape(self.n_cores, *self.out_avals[i].shape)[c]
                for i, n in enumerate(self.out_names)})
        return results, best


# ======================================================================
# kernel entry
# ======================================================================
_CACHE = {}


def kernel(**inputs):
    x = np.asarray(inputs['x'], dtype=np.float32)   # [128, 8000, 1]
    n_cores = 8
    W = prep_weights(inputs)

    shared = {
        'lhsT1': W['lhsT1'], 'b1c': W['b1c'].reshape(20, 1),
        'lhsT2': W['lhsT2'], 'b2c': W['b2c'].reshape(80, 1),
        'lhsT3': W['lhsT3'], 'b3c': np.ascontiguousarray(W['b3c'].T),
        'WiT': W['WiT_dev'], 'WhT': W['WhT_dev'],
        'bLc': np.ascontiguousarray(W['bLc'].transpose(2, 0, 1)),
        'dWT': W['dWT_dev'], 'dbc': W['dbc'].reshape(5, 1),
        'ident': np.eye(128, dtype=np.float16),
    }
    in_maps = []
    for c in range(n_cores):
        m = dict(shared)
        m['rhs1'] = prep_x(x[c * B:(c + 1) * B, :, 0])
        in_maps.append(m)

    if 'runner' not in _CACHE:
        nc = build(layers=5, with_conv=True)
        split_excess_waits(nc)
        _CACHE['runner'] = SpmdRunner(nc, n_cores)
    runner = _CACHE['runner']
    in_dev = runner.place_inputs(in_maps)
    results, best = runner.run(in_dev, time_reps=int(_CACHE.get('reps', 0)))
    _CACHE['last_time_s'] = best

    out = np.empty((128, 1600, 5), np.float32)
    for c in range(n_cores):
        out[c * B:(c + 1) * B] = results[c]['y'].transpose(2, 1, 0)
    return out

